# revision 1
# baseline (speedup 1.0000x reference)
"""Trainium2 Bass kernel for the MoE-routing module.

Computation (B=32768, D=1024, H=512, F=100, E=16, K=2):
    h   = relu(x @ W_shared + b_shared)                  [B, H]
    a   = relu(einsum('bh,ehf', h, W1) + b1)             [B, E, F]
    o   = einsum('bef,efo', a, W2) + b2                  [B, E, 1]
    out = mean over the K routed experts of o[b, send_to[idx[b]]]

Strategy: host sorts tokens by head id and shards the sorted batch over the
8 cores (4096 tokens each, perfectly balanced).  A sorted 4096-token window
only routes to a handful of consecutive experts, so each core gets just the
expert slices it needs (EC slots, adaptively >= actual need; EC=16 degrades
to the dense all-expert kernel).  Routing is folded into a host-computed
per-slot mask M[j, b], so the device computes
    out[b] = sum_j o_local[b, j] * M[j, b]
with three matmul stages, features on SBUF partitions throughout:
  M1: hT[h, t]  = relu(W_shared.T @ xT)         lhsT = W_shared tiles
  M2: aT[f', t] = relu(W1sel.T @ hT)            f' = j*F + f  (EC*F wide)
  M3: c[j, t]   = W2sel.T @ aT                  W2sel block-diagonal
  sel: out[t]   = ones.T @ (c * mask)           1-partition result row
All matmuls run as float32r (full-rate fp32 mode, ~1e-4 rel err).
"""

import os

import numpy as np

import concourse.mybir as mybir
from concourse import bacc
from concourse.bass_utils import run_bass_kernel_spmd
from concourse.tile import TileContext

B, D, H, F, E, TOPK = 32768, 1024, 512, 100, 16, 2
N_CORES = 8
BL = B // N_CORES          # tokens per core
CHUNK = 512                # tokens per device-side tile loop
N_CHUNKS = BL // CHUNK
MH = H // 128              # M1 output tiles
KD = D // 128              # M1 contraction tiles
KH = H // 128              # M2 contraction tiles
EC_MIN = 5                 # minimum expert slots per core
CHUNK_SIZES = [512] * 8

# Compute dtype for the matmul stages: "float32", "float32r", or "bfloat16"
COMPUTE_DT = os.environ.get("KERNEL_DT", "float32r")

_FP32 = mybir.dt.float32
_cache = {}


def _np_in_dtype():
    import ml_dtypes

    return ml_dtypes.bfloat16 if COMPUTE_DT == "bfloat16" else np.float32


def _build_nc(ec):
    """Build the SPMD program for EC expert slots per core."""
    CDT = getattr(mybir.dt, COMPUTE_DT)
    SDT = mybir.dt.bfloat16 if COMPUTE_DT == "bfloat16" else mybir.dt.float32
    EF = ec * F                    # local expert-concat width
    KT3 = (EF + 127) // 128        # M2 output tiles / M3 contraction tiles
    EF_PAD = KT3 * 128             # w1sel zero-padded so all tiles are full
    NB = MH + KT3 + 1              # packed bias columns

    nc = bacc.Bacc("TRN2", target_bir_lowering=False, num_devices=N_CORES)

    xT_d = nc.declare_dram_parameter("xT", [D * BL], CDT, isOutput=False)
    mask_d = nc.declare_dram_parameter("mask", [33, BL], _FP32, isOutput=False)
    wsh_d = nc.declare_dram_parameter("wsh", [D, H], CDT, isOutput=False)
    w1c_d = nc.declare_dram_parameter("w1c", [H, EF_PAD], CDT, isOutput=False)
    w2bd_d = nc.declare_dram_parameter("w2bd", [128, KT3 * ec], CDT, isOutput=False)
    bias_d = nc.declare_dram_parameter("biases", [128, NB], _FP32, isOutput=False)
    out_d = nc.declare_dram_parameter("out", [BL], _FP32, isOutput=True)

    relu = mybir.ActivationFunctionType.Relu
    sizes = CHUNK_SIZES
    offs = np.cumsum([0] + sizes).tolist()

    with TileContext(nc) as tc:
        with (
            tc.tile_pool(name="weights", bufs=1) as wpool,
            tc.tile_pool(name="xin", bufs=3) as xpool,
            tc.tile_pool(name="mid", bufs=3) as midpool,
            tc.tile_pool(name="small", bufs=3) as spool,
            tc.tile_pool(name="ps_h", bufs=4, space="PSUM") as ps_h,
            tc.tile_pool(name="ps_a", bufs=2, space="PSUM") as ps_a,
            tc.tile_pool(name="ps_c", bufs=1, space="PSUM") as ps_c,
            tc.tile_pool(name="ps_o", bufs=1, space="PSUM") as ps_o,
        ):
            # ---- input DMAs: explicit priorities pin queue order to
            # program order.  Separate tiles per k-piece — Tile dependency
            # tracking is per-tile, so split DMAs into one tile would
            # serialize as write-after-write.  wsh + chunk-0 x interleave
            # across both HWDGE queues so M1 starts after the first ~512KB.
            _prio = [0]

            def pdma(q, dst, src):
                inst = q.dma_start(dst, src)
                inst.ins.bass_priority = _prio[0]
                _prio[0] += 1
                return inst

            def xview(c):
                sz = sizes[c]
                o = offs[c] * D
                return xT_d[o : o + D * sz].rearrange("(ko p t) -> p ko t", p=128, t=sz)

            wsh_view = wsh_d.rearrange("(o p) h -> p o h", p=128)
            wsh_ks = [wpool.tile([128, H], CDT, name=f"wshk{k}") for k in range(KD)]
            xt0_view = xview(0)
            xt0 = [
                xpool.tile([128, CHUNK], CDT, tag=f"xt{k}", name=f"xt0_{k}")
                for k in range(KD)
            ]
            for k in range(KD):
                qa = nc.sync if k % 2 == 0 else nc.scalar
                qb = nc.scalar if k % 2 == 0 else nc.sync
                pdma(qa, wsh_ks[k][:], wsh_view[:, k])
                pdma(qb, xt0[k][:, : sizes[0]], xt0_view[:, k])

            xts, masks = [[t[:, : sizes[0]] for t in xt0]], []
            w1c_ks = [None] * KH
            for c in range(len(sizes)):
                sz = sizes[c]
                if c > 0:
                    xv = xview(c)
                    xa = xpool.tile([128, KD // 2, CHUNK], CDT, tag="xta", name=f"xta{c}")
                    xb = xpool.tile([128, KD // 2, CHUNK], CDT, tag="xtb", name=f"xtb{c}")
                    pdma(nc.scalar, xa[:, :, :sz], xv[:, : KD // 2])
                    pdma(nc.sync, xb[:, :, :sz], xv[:, KD // 2 :])
                    xts.append([xa[:, k, :sz] for k in range(KD // 2)] + [xb[:, k, :sz] for k in range(KD // 2)])
                mask_sb = spool.tile([33, CHUNK], _FP32, tag="mask")
                pdma(nc.scalar, mask_sb[:, :sz], mask_d[:, offs[c] : offs[c] + sz])
                masks.append(mask_sb[:, :sz])
                if c == 0:
                    w1c_view = w1c_d.rearrange("(o p) f -> p o f", p=128)
                    for k in range(KH):
                        w1c_ks[k] = wpool.tile([128, EF_PAD], CDT, name=f"w1ck{k}")
                        pdma(nc.sync if k % 2 == 0 else nc.scalar, w1c_ks[k][:], w1c_view[:, k])
                    w2bd_sb = wpool.tile([128, KT3 * ec], CDT)
                    pdma(nc.sync, w2bd_sb[:], w2bd_d[:])
                    bias_sb = wpool.tile([128, NB], _FP32)
                    pdma(nc.sync, bias_sb[:], bias_d[:])
                    ones_sb = wpool.tile([ec, 1], CDT)
                    if COMPUTE_DT == "float32r":
                        nc.vector.memset(ones_sb[:].bitcast(mybir.dt.float32), 1.0)
                    else:
                        nc.vector.memset(ones_sb[:], 1.0)

            for c in range(len(sizes)):
                sz = sizes[c]
                t0 = offs[c]
                xt = xts[c]
                mask_sb = masks[c]

                # ---- M1: hT = relu(W_shared.T @ xT + b) ----
                # chunk 0 runs k-outer so matmuls start as soon as the first
                # split DMA pieces land; later chunks are fully prefetched.
                hT = midpool.tile([128, MH, CHUNK], CDT, tag="hT", name=f"hT{c}")[:, :, :sz]
                if c == 0:
                    phs = [ps_h.tile([128, CHUNK], _FP32, tag="ps_h", name=f"ph{m}")[:, :sz] for m in range(MH)]
                    for k in range(KD):
                        for m in range(MH):
                            nc.tensor.matmul(
                                phs[m][:],
                                lhsT=wsh_ks[k][:, m * 128 : (m + 1) * 128],
                                rhs=xt[k][:],
                                start=(k == 0),
                                stop=(k == KD - 1),
                            )
                    for m in range(MH):
                        nc.scalar.activation(
                            hT[:, m, :], phs[m][:], relu, bias=bias_sb[:, m : m + 1]
                        )
                else:
                    for m in range(MH):
                        ph = ps_h.tile([128, CHUNK], _FP32, tag="ps_h", name=f"phx{c}_{m}")[:, :sz]
                        for k in range(KD):
                            nc.tensor.matmul(
                                ph[:],
                                lhsT=wsh_ks[k][:, m * 128 : (m + 1) * 128],
                                rhs=xt[k][:],
                                start=(k == 0),
                                stop=(k == KD - 1),
                            )
                        nc.scalar.activation(
                            hT[:, m, :], ph[:], relu, bias=bias_sb[:, m : m + 1]
                        )

                # ---- M2: aT = relu(W1sel.T @ hT + b1) ----
                aT = midpool.tile([128, KT3, CHUNK], CDT, tag="aT", name=f"aT{c}")[:, :, :sz]
                for m in range(KT3):
                    f0 = m * 128
                    pa = ps_a.tile([128, CHUNK], _FP32, tag="ps_a", name=f"pa{c}_{m}")[:, :sz]
                    for k in range(KH):
                        nc.tensor.matmul(
                            pa[:],
                            lhsT=w1c_ks[k][:, f0 : f0 + 128],
                            rhs=hT[:, k, :],
                            start=(k == 0),
                            stop=(k == KH - 1),
                        )
                    nc.scalar.activation(
                        aT[:, m, :], pa[:], relu,
                        bias=bias_sb[:, MH + m : MH + m + 1],
                    )

                # ---- M3: c = W2sel.T @ aT  (block-diag W2) ----
                pc = ps_c.tile([ec, CHUNK], _FP32, tag="ps_c", name=f"pc{c}")[:, :sz]
                for k in range(KT3):
                    nc.tensor.matmul(
                        pc[:],
                        lhsT=w2bd_sb[:, k * ec : (k + 1) * ec],
                        rhs=aT[:, k, :],
                        start=(k == 0),
                        stop=(k == KT3 - 1),
                    )

                # ---- select: out = ones.T @ (c * mask) + btok ----
                msel = spool.tile([ec, CHUNK], CDT, tag="msel", name=f"msel{c}")[:, :sz]
                nc.vector.tensor_mul(msel[:], pc[:], mask_sb[:ec])
                po = ps_o.tile([1, CHUNK], _FP32, tag="ps_o", name=f"po{c}")[:, :sz]
                nc.tensor.matmul(po[:], lhsT=ones_sb[:], rhs=msel[:], start=True, stop=True)
                ot = spool.tile([1, CHUNK], _FP32, tag="ot", name=f"ot{c}")[:, :sz]
                nc.vector.tensor_add(ot[:], po[:], mask_sb[32:33])
                nc.gpsimd.dma_start(out_d[t0 : t0 + sz].rearrange("(o t) -> o t", o=1), ot[:])

    nc.compile()
    return nc


def get_nc(ec):
    key = (COMPUTE_DT, ec)
    if key not in _cache:
        _cache[key] = _build_nc(ec)
    return _cache[key]


def prepare(inputs):
    """Host-side routing/sorting/sharding. Returns (ec, in_maps, perm)."""
    np_dt = _np_in_dtype()
    x = np.asarray(inputs["x"], dtype=np.float32)
    idx = np.asarray(inputs["idx"]).astype(np.int64).reshape(B)
    W_shared = np.asarray(inputs["W_shared"], dtype=np.float32)
    b_shared = np.asarray(inputs["b_shared"], dtype=np.float32).reshape(H)
    W1 = np.asarray(inputs["W1"], dtype=np.float32)
    b1 = np.asarray(inputs["b1"], dtype=np.float32).reshape(E, F)
    W2 = np.asarray(inputs["W2"], dtype=np.float32).reshape(E, F)
    b2 = np.asarray(inputs["b2"], dtype=np.float32).reshape(E)
    send_to = np.asarray(inputs["send_to"]).astype(np.int64)

    perm = np.argsort(idx, kind="stable")
    idx_s = idx[perm]
    routes_s = send_to[idx_s]                      # [B, K] sorted routes
    x_s = x[perm]                                  # [B, D]

    # per-core expert lists
    expert_lists = []
    for c in range(N_CORES):
        sl = slice(c * BL, (c + 1) * BL)
        expert_lists.append(np.unique(routes_s[sl]))
    ec = max(EC_MIN, max(len(el) for el in expert_lists))
    ec = min(ec, E)

    wsh = np.ascontiguousarray(W_shared).astype(np_dt)
    EF = ec * F
    KT3 = (EF + 127) // 128
    EF_PAD = KT3 * 128
    NB = MH + KT3 + 1

    in_maps = []
    for c in range(N_CORES):
        sl = slice(c * BL, (c + 1) * BL)
        el = expert_lists[c]
        # local slot tables (pad slots use sentinel -1: zero weights, no mask)
        slots = np.full(ec, -1, dtype=np.int64)
        slots[: len(el)] = el

        # mask[j, b] = (1/K) * count of slots[j] among routes of token b
        r = routes_s[sl]                            # [BL, K]
        mask = np.zeros((33, BL), dtype=np.float32)
        for k in range(r.shape[1]):
            hit = slots[:, None] == r[None, :, k]   # [ec, BL]
            mask[:ec] += hit.astype(np.float32) / r.shape[1]
        mask[32] = b2[r].mean(axis=1)               # routed-b2 mean per token

        w1sel = np.zeros((H, EF_PAD), dtype=np.float32)
        b1sel = np.zeros(EF_PAD, dtype=np.float32)
        w2full = np.zeros((EF_PAD, ec), dtype=np.float32)
        for j, e in enumerate(slots):
            if e < 0:
                continue
            w1sel[:, j * F : (j + 1) * F] = W1[e]
            b1sel[j * F : (j + 1) * F] = b1[e]
            w2full[j * F : (j + 1) * F, j] = W2[e]
        w2bd = np.ascontiguousarray(
            w2full.reshape(KT3, 128, ec).transpose(1, 0, 2).reshape(128, KT3 * ec)
        ).astype(np_dt)

        biases = np.zeros((128, NB), dtype=np.float32)
        biases[:, :MH] = b_shared.reshape(MH, 128).T
        biases[:, MH : MH + KT3] = b1sel.reshape(KT3, 128).T
        biases[:ec, MH + KT3] = b2[np.maximum(slots, 0)] * (slots >= 0)

        xc = x_s[sl]
        parts, o = [], 0
        for szc in CHUNK_SIZES:
            parts.append(xc[o : o + szc].T.ravel())
            o += szc
        xT = np.ascontiguousarray(np.concatenate(parts)).astype(np_dt)

        in_maps.append(
            {
                "xT": xT,
                "mask": mask,
                "wsh": wsh,
                "w1c": w1sel.astype(np_dt),
                "w2bd": w2bd,
                "biases": biases,
            }
        )
    return ec, in_maps, perm


def kernel(**inputs) -> np.ndarray:
    ec, in_maps, perm = prepare(inputs)
    nc = get_nc(ec)
    res = run_bass_kernel_spmd(nc, in_maps, list(range(N_CORES)))
    out_sorted = np.concatenate([res.results[c]["out"] for c in range(N_CORES)])
    out = np.empty(B, dtype=np.float32)
    out[perm] = out_sorted
    return out.reshape(B, 1)



# revision 5
# speedup vs baseline: 1.2504x; 1.2504x over previous
"""Trainium2 Bass kernel for the MoE-routing module.

Computation (B=32768, D=1024, H=512, F=100, E=16, K=2):
    h   = relu(x @ W_shared + b_shared)                  [B, H]
    a   = relu(einsum('bh,ehf', h, W1) + b1)             [B, E, F]
    o   = einsum('bef,efo', a, W2) + b2                  [B, E, 1]
    out = mean over the K routed experts of o[b, send_to[idx[b]]]

Strategy: host sorts tokens by head id and shards the sorted batch over
the 8 cores (4096 tokens each), 8 device chunks of 512 per core.  A head
group covers ~4 chunks, so most chunk positions hold a single head id on
every core: those need exactly the 2 routed experts, and the top-2 mean
collapses to a constant 0.5/0.5 blend that is folded into W2 — the
select stage merges into M3 as a 1-column matmul (no mask, no vector
work).  Positions where any core crosses a head boundary run a general
masked path with EC slots (3 normally).  Per-position structure (slot
count + masked?) is uniform across cores, so one SPMD program serves all
8; programs are cached per structure key.

All matmuls run in fp16: same 1 cycle/row PE rate as fp32r at 512-wide
moving tiles, but half the HBM traffic; final rel err ~6e-4 (fp8 was
measured at 4e-2 — over the 2e-2 budget — and is not used).

Stages (features on SBUF partitions throughout):
  M1: hT[h, t]  = relu(W_shared.T @ xT + b)       8 k-tiles as 4 pairs
  M2: aT[f', t] = relu(W1sel.T @ hT + b1)         f' = slot*128 + f
  M3 single: out[t] = 0.5*(W2cat).T @ aT + b2m    1-col lhsT, merged sel
  M3 mixed:  c[j, t] = W2bd.T @ aT; out = ones.T @ (c * mask) + b2m[t]
"""

import numpy as np

import concourse.mybir as mybir
from concourse import bacc
from concourse.bass_utils import run_bass_kernel_spmd
from concourse.tile import TileContext

B, D, H, F, E, TOPK = 32768, 1024, 512, 100, 16, 2
N_CORES = 8
BL = B // N_CORES          # tokens per core
CHUNK = 512                # tokens per device-side tile loop
NCH = BL // CHUNK          # 8 chunks per core
KD = D // 128              # M1 contraction tiles
NPAIR = KD // 2            # M1 contraction tile pairs (DMA granularity)
MH = H // 128              # M1 output tiles
KH = H // 128              # M2 contraction tiles

COMPUTE_DT = "float16"
CDT = mybir.dt.float16
NP_CDT = np.float16
_FP32 = mybir.dt.float32
_cache = {}


def _build_nc(key):
    """Build the SPMD program for per-position (slot count, masked) key."""
    ecs, mixed = key
    n_mixed = sum(mixed)
    max_ec = max(ecs)
    MROWS = 33                           # mask rows: slots + b2mean at row 32
    W2COLS = sum(e * e if mx else e for e, mx in zip(ecs, mixed))
    NB = MH + sum(ecs) + sum(0 if mx else 1 for mx in mixed)

    nc = bacc.Bacc("TRN2", target_bir_lowering=False, num_devices=N_CORES)

    xT_d = nc.declare_dram_parameter("xT", [D * BL], CDT, isOutput=False)
    wsh_d = nc.declare_dram_parameter("wsh", [D * H], CDT, isOutput=False)
    w1sz = [KH * 128 * e * 128 for e in ecs]
    w1off = np.cumsum([0] + w1sz).tolist()
    w1c_d = nc.declare_dram_parameter("w1c", [w1off[-1]], CDT, isOutput=False)
    w2_d = nc.declare_dram_parameter("w2", [128, W2COLS], CDT, isOutput=False)
    bias_d = nc.declare_dram_parameter("biases", [128, NB], _FP32, isOutput=False)
    if n_mixed:
        mask_d = nc.declare_dram_parameter(
            "mask", [MROWS, n_mixed * CHUNK], _FP32, isOutput=False
        )
    out_d = nc.declare_dram_parameter("out", [BL], _FP32, isOutput=True)

    relu = mybir.ActivationFunctionType.Relu
    copyf = mybir.ActivationFunctionType.Identity

    with TileContext(nc) as tc:
        with (
            tc.tile_pool(name="weights", bufs=1) as wpool,
            tc.tile_pool(name="xin", bufs=16) as xpool,
            tc.tile_pool(name="w1p", bufs=3) as w1pool,
            tc.tile_pool(name="hmid", bufs=3) as hpool,
            tc.tile_pool(name="amid", bufs=3) as apool,
            tc.tile_pool(name="small", bufs=4) as spool,
            tc.tile_pool(name="ps_h", bufs=4, space="PSUM") as ps_h,
            tc.tile_pool(name="ps_a", bufs=2, space="PSUM") as ps_a,
            tc.tile_pool(name="ps_c", bufs=1, space="PSUM") as ps_c,
            tc.tile_pool(name="ps_o", bufs=1, space="PSUM") as ps_o,
        ):
            # ---- DMAs with explicit priorities pinning queue order ----
            _prio = [0]

            def pdma(q, dst, src):
                inst = q.dma_start(dst, src)
                inst.ins.bass_priority = _prio[0]
                _prio[0] += 1
                return inst

            def xpair_view(j, p):
                o = (j * NPAIR + p) * (128 * 2 * CHUNK)
                return xT_d[o : o + 128 * 2 * CHUNK].rearrange(
                    "(p i t) -> p i t", p=128, i=2
                )

            def wsh_view(p):
                o = p * (128 * 2 * H)
                return wsh_d[o : o + 128 * 2 * H].rearrange(
                    "(p i h) -> p i h", p=128, i=2
                )

            def w1_view(j):
                return w1c_d[w1off[j] : w1off[j + 1]].rearrange(
                    "(p k c) -> p k c", p=128, k=KH
                )

            # head: wsh pairs + x chunk-0 pairs interleaved on both queues
            wshp = [wpool.tile([128, 2, H], CDT, name=f"wshp{p}") for p in range(NPAIR)]
            xts = [
                [
                    xpool.tile([128, 2, CHUNK], CDT, tag="xt", name=f"xt{j}_{p}")
                    for p in range(NPAIR)
                ]
                for j in range(NCH)
            ]
            for p in range(NPAIR):
                qa = nc.sync if p % 2 == 0 else nc.scalar
                qb = nc.scalar if p % 2 == 0 else nc.sync
                pdma(qa, wshp[p][:], wsh_view(p))
                pdma(qb, xts[0][p][:], xpair_view(0, p))

            w1sb = [None] * NCH
            w1sb[0] = w1pool.tile([128, KH, ecs[0] * 128], CDT, tag="w1", name="w1_0")
            pdma(nc.sync, w1sb[0][:], w1_view(0))
            bias_sb = wpool.tile([128, NB], _FP32)
            pdma(nc.scalar, bias_sb[:], bias_d[:])
            w2_sb = wpool.tile([128, W2COLS], CDT)
            pdma(nc.sync, w2_sb[:], w2_d[:])
            if n_mixed:
                mask_sb = wpool.tile([MROWS, n_mixed * CHUNK], _FP32)
                pdma(nc.scalar, mask_sb[:], mask_d[:])
                ones_sb = wpool.tile([max_ec, 1], CDT)
                nc.vector.memset(ones_sb[:], 1.0)

            for j in range(1, NCH):
                for p in range(NPAIR):
                    pdma(
                        nc.sync if p % 2 == 0 else nc.scalar,
                        xts[j][p][:],
                        xpair_view(j, p),
                    )
                w1sb[j] = w1pool.tile(
                    [128, KH, ecs[j] * 128], CDT, tag="w1", name=f"w1_{j}"
                )
                pdma(nc.scalar if j % 2 else nc.sync, w1sb[j][:], w1_view(j))

            # ---- compute ----
            w2c = 0          # running col offset into w2_sb
            bcol = MH        # running col offset into bias_sb
            mj = 0           # mixed-chunk ordinal
            for j in range(NCH):
                ec = ecs[j]
                xt = xts[j]

                # M1: hT = relu(W_shared.T @ xT + b)
                hT = hpool.tile([128, MH, CHUNK], CDT, tag="hT", name=f"hT{j}")
                if j == 0:
                    # k-outer so matmuls start as the first pair DMAs land
                    phs = [
                        ps_h.tile([128, CHUNK], _FP32, tag="ps_h", name=f"ph{m}")
                        for m in range(MH)
                    ]
                    for p in range(NPAIR):
                        for i in range(2):
                            for m in range(MH):
                                nc.tensor.matmul(
                                    phs[m][:],
                                    lhsT=wshp[p][:, i, m * 128 : (m + 1) * 128],
                                    rhs=xt[p][:, i, :],
                                    start=(p == 0 and i == 0),
                                    stop=(p == NPAIR - 1 and i == 1),
                                )
                    for m in range(MH):
                        nc.scalar.activation(
                            hT[:, m, :], phs[m][:], relu, bias=bias_sb[:, m : m + 1]
                        )
                else:
                    for m in range(MH):
                        ph = ps_h.tile(
                            [128, CHUNK], _FP32, tag="ps_h", name=f"ph{j}_{m}"
                        )
                        for p in range(NPAIR):
                            for i in range(2):
                                nc.tensor.matmul(
                                    ph[:],
                                    lhsT=wshp[p][:, i, m * 128 : (m + 1) * 128],
                                    rhs=xt[p][:, i, :],
                                    start=(p == 0 and i == 0),
                                    stop=(p == NPAIR - 1 and i == 1),
                                )
                        nc.scalar.activation(
                            hT[:, m, :], ph[:], relu, bias=bias_sb[:, m : m + 1]
                        )

                # M2: aT = relu(W1sel.T @ hT + b1)
                aT = apool.tile([128, ec, CHUNK], CDT, tag="aT", name=f"aT{j}")
                for mi in range(ec):
                    pa = ps_a.tile([128, CHUNK], _FP32, tag="ps_a", name=f"pa{j}_{mi}")
                    for k in range(KH):
                        nc.tensor.matmul(
                            pa[:],
                            lhsT=w1sb[j][:, k, mi * 128 : (mi + 1) * 128],
                            rhs=hT[:, k, :],
                            start=(k == 0),
                            stop=(k == KH - 1),
                        )
                    nc.scalar.activation(
                        aT[:, mi, :], pa[:], relu,
                        bias=bias_sb[:, bcol + mi : bcol + mi + 1],
                    )

                # M3 + select
                t0 = j * CHUNK
                ot = spool.tile([1, CHUNK], _FP32, tag="ot", name=f"ot{j}")
                if not mixed[j]:
                    po = ps_o.tile([1, CHUNK], _FP32, tag="ps_o", name=f"po{j}")
                    for k in range(ec):
                        nc.tensor.matmul(
                            po[:],
                            lhsT=w2_sb[:, w2c + k : w2c + k + 1],
                            rhs=aT[:, k, :],
                            start=(k == 0),
                            stop=(k == ec - 1),
                        )
                    nc.scalar.activation(
                        ot[:], po[:], copyf,
                        bias=bias_sb[0:1, bcol + ec : bcol + ec + 1],
                    )
                    w2c += ec
                    bcol += ec + 1
                else:
                    pc = ps_c.tile([ec, CHUNK], _FP32, tag="ps_c", name=f"pc{j}")
                    for k in range(ec):
                        nc.tensor.matmul(
                            pc[:],
                            lhsT=w2_sb[:, w2c + k * ec : w2c + (k + 1) * ec],
                            rhs=aT[:, k, :],
                            start=(k == 0),
                            stop=(k == ec - 1),
                        )
                    msel = spool.tile([ec, CHUNK], CDT, tag="msel", name=f"msel{j}")
                    nc.vector.tensor_mul(
                        msel[:], pc[:], mask_sb[:ec, mj * CHUNK : (mj + 1) * CHUNK]
                    )
                    po = ps_o.tile([1, CHUNK], _FP32, tag="ps_o", name=f"pom{j}")
                    nc.tensor.matmul(
                        po[:], lhsT=ones_sb[:ec, :], rhs=msel[:], start=True, stop=True
                    )
                    nc.vector.tensor_add(
                        ot[:], po[:],
                        mask_sb[32:33, mj * CHUNK : (mj + 1) * CHUNK],
                    )
                    w2c += ec * ec
                    bcol += ec
                    mj += 1
                nc.gpsimd.dma_start(
                    out_d[t0 : t0 + CHUNK].rearrange("(o t) -> o t", o=1), ot[:]
                )

    nc.compile()
    return nc


def get_nc(key):
    if key not in _cache:
        _cache[key] = _build_nc(key)
    return _cache[key]


def prepare(inputs):
    """Host-side routing/sorting/sharding. Returns (key, in_maps, perm)."""
    x = np.asarray(inputs["x"], dtype=np.float32)
    idx = np.asarray(inputs["idx"]).astype(np.int64).reshape(B)
    W_shared = np.asarray(inputs["W_shared"], dtype=np.float32)
    b_shared = np.asarray(inputs["b_shared"], dtype=np.float32).reshape(H)
    W1 = np.asarray(inputs["W1"], dtype=np.float32)
    b1 = np.asarray(inputs["b1"], dtype=np.float32).reshape(E, F)
    W2 = np.asarray(inputs["W2"], dtype=np.float32).reshape(E, F)
    b2 = np.asarray(inputs["b2"], dtype=np.float32).reshape(E)
    send_to = np.asarray(inputs["send_to"]).astype(np.int64)

    perm = np.argsort(idx, kind="stable")
    idx_s = idx[perm]
    x_s = x[perm]
    routes_s = send_to[idx_s]                      # [B, K] sorted routes

    # per-position structure: slot count + masked?, uniform across cores
    slot_lists = [[None] * NCH for _ in range(N_CORES)]
    ecs, mixed = [], []
    for j in range(NCH):
        ec_j, mx_j = 2, False
        for c in range(N_CORES):
            sl = slice(c * BL + j * CHUNK, c * BL + (j + 1) * CHUNK)
            experts = np.unique(routes_s[sl])
            slot_lists[c][j] = experts
            ec_j = max(ec_j, len(experts))
            if len(np.unique(idx_s[sl])) > 1:
                mx_j = True
        ecs.append(ec_j)
        mixed.append(mx_j)
    ecs, mixed = tuple(ecs), tuple(mixed)
    n_mixed = sum(mixed)
    max_ec = max(ecs)
    MROWS = 33

    # wsh pair blocks [pair, 128, 2, H]
    wshr = W_shared.reshape(NPAIR, 2, 128, H)
    wsh_flat = np.ascontiguousarray(wshr.transpose(0, 2, 1, 3)).astype(NP_CDT).ravel()

    in_maps = []
    for c in range(N_CORES):
        xc = x_s[c * BL : (c + 1) * BL]
        # x pair blocks [chunk, pair, 128, 2, CHUNK]
        xcr = xc.reshape(NCH, CHUNK, NPAIR, 2, 128)
        xT = np.ascontiguousarray(xcr.transpose(0, 2, 4, 3, 1)).astype(NP_CDT).ravel()

        w1_parts = []
        w2_cols = []
        bias_cols = [b_shared.reshape(MH, 128).T]
        mask_cols = np.zeros((MROWS, max(n_mixed, 1) * CHUNK), np.float32)
        mj = 0
        for j in range(NCH):
            sl = slice(c * BL + j * CHUNK, c * BL + (j + 1) * CHUNK)
            ec = ecs[j]
            slots = np.full(ec, -1, dtype=np.int64)
            el = slot_lists[c][j]
            slots[: len(el)] = el

            w1sel = np.zeros((H, ec * 128), np.float32)
            b1sel = np.zeros(ec * 128, np.float32)
            for mi, e in enumerate(slots):
                if e < 0:
                    continue
                w1sel[:, mi * 128 : mi * 128 + F] = W1[e]
                b1sel[mi * 128 : mi * 128 + F] = b1[e]
            w1_parts.append(
                np.ascontiguousarray(
                    w1sel.reshape(KH, 128, ec * 128).transpose(1, 0, 2)
                ).astype(NP_CDT).ravel()
            )
            bias_cols.append(b1sel.reshape(ec, 128).T)

            r = routes_s[sl]                        # [CHUNK, K]
            if not mixed[j]:
                w2m = np.zeros((128, ec), np.float32)
                for e in r[0]:  # routes with multiplicity
                    mi = int(np.where(slots == e)[0][0])
                    w2m[:F, mi] += W2[e] / r.shape[1]
                w2_cols.append(w2m)
                col = np.zeros((128, 1), np.float32)
                col[0, 0] = b2[r[0]].mean()
                bias_cols.append(col)
            else:
                w2full = np.zeros((ec * 128, ec), np.float32)
                for mi, e in enumerate(slots):
                    if e < 0:
                        continue
                    w2full[mi * 128 : mi * 128 + F, mi] = W2[e]
                w2_cols.append(
                    w2full.reshape(ec, 128, ec).transpose(1, 0, 2).reshape(128, ec * ec)
                )
                for k in range(r.shape[1]):
                    hit = slots[:, None] == r[None, :, k]
                    mask_cols[:ec, mj * CHUNK : (mj + 1) * CHUNK] += (
                        hit.astype(np.float32) / r.shape[1]
                    )
                mask_cols[32, mj * CHUNK : (mj + 1) * CHUNK] = b2[r].mean(axis=1)
                mj += 1

        in_map = {
            "xT": xT,
            "wsh": wsh_flat,
            "w1c": np.concatenate(w1_parts),
            "w2": np.concatenate(w2_cols, axis=1).astype(NP_CDT),
            "biases": np.ascontiguousarray(
                np.concatenate(bias_cols, axis=1)
            ).astype(np.float32),
        }
        if n_mixed:
            in_map["mask"] = mask_cols
        in_maps.append(in_map)
    return (ecs, mixed), in_maps, perm


def kernel(**inputs) -> np.ndarray:
    key, in_maps, perm = prepare(inputs)
    nc = get_nc(key)
    res = run_bass_kernel_spmd(nc, in_maps, list(range(N_CORES)))
    out_sorted = np.concatenate([res.results[c]["out"] for c in range(N_CORES)])
    out = np.empty(B, dtype=np.float32)
    out[perm] = out_sorted
    return out.reshape(B, 1)


# revision 9
# speedup vs baseline: 1.2576x; 1.0057x over previous
"""Trainium2 Bass kernel for the MoE-routing module.

Computation (B=32768, D=1024, H=512, F=100, E=16, K=2):
    h   = relu(x @ W_shared + b_shared)                  [B, H]
    a   = relu(einsum('bh,ehf', h, W1) + b1)             [B, E, F]
    o   = einsum('bef,efo', a, W2) + b2                  [B, E, 1]
    out = mean over the K routed experts of o[b, send_to[idx[b]]]

Strategy: host sorts tokens by head id and shards the sorted batch over
the 8 cores (4096 tokens each), 8 device chunks of 512 per core.  A head
group covers ~4 chunks, so most chunk positions hold a single head id on
every core: those need exactly the 2 routed experts, and the top-2 mean
collapses to a constant 0.5/0.5 blend that is folded into W2 — the
select stage merges into M3 as a 1-column matmul (no mask, no vector
work).  Positions where any core crosses a head boundary run a general
masked path with EC slots (3 normally).  Per-position structure (slot
count + masked?) is uniform across cores, so one SPMD program serves all
8; programs are cached per structure key.

All matmuls run in fp16: same 1 cycle/row PE rate as fp32r at 512-wide
moving tiles, but half the HBM traffic; final rel err ~6e-4 (fp8 was
measured at 4e-2 — over the 2e-2 budget — and is not used).

Stages (features on SBUF partitions throughout):
  M1: hT[h, t]  = relu(W_shared.T @ xT + b)       8 k-tiles as 4 pairs
  M2: aT[f', t] = relu(W1sel.T @ hT + b1)         f' = slot*128 + f
  M3 single: out[t] = 0.5*(W2cat).T @ aT + b2m    1-col lhsT, merged sel
  M3 mixed:  c[j, t] = W2bd.T @ aT; out = ones.T @ (c * mask) + b2m[t]
"""

import numpy as np

import concourse.mybir as mybir
from concourse import bacc
from concourse.bass_utils import run_bass_kernel_spmd
from concourse.tile import TileContext

B, D, H, F, E, TOPK = 32768, 1024, 512, 100, 16, 2
N_CORES = 8
BL = B // N_CORES          # tokens per core
CHUNK = 512                # max tokens per device-side tile loop
# smaller head chunks let M1 start while the DMA backlog clears
SIZES = (256, 256, 512, 512, 512, 512, 512, 512, 512)
OFFS = [0]
for _s in SIZES:
    OFFS.append(OFFS[-1] + _s)
assert OFFS[-1] == BL
NCH = len(SIZES)           # chunks per core
KD = D // 128              # M1 contraction tiles
NPAIR = KD // 2            # M1 contraction tile pairs (DMA granularity)
MH = H // 128              # M1 output tiles
KH = H // 128              # M2 contraction tiles

COMPUTE_DT = "float16"
CDT = mybir.dt.float16
NP_CDT = np.float16
_FP32 = mybir.dt.float32
_cache = {}


def _build_nc(key):
    """Build the SPMD program for per-position (slot count, masked) key."""
    ecs, mixed = key
    n_mixed = sum(mixed)
    max_ec = max(ecs)
    MROWS = 33                           # mask rows: slots + b2mean at row 32
    W2COLS = sum(e * e if mx else e for e, mx in zip(ecs, mixed))
    NB = MH + sum(ecs) + sum(0 if mx else 1 for mx in mixed)

    nc = bacc.Bacc("TRN2", target_bir_lowering=False, num_devices=N_CORES)

    xT_d = nc.declare_dram_parameter("xT", [D * BL], CDT, isOutput=False)
    wsh_d = nc.declare_dram_parameter("wsh", [D * H], CDT, isOutput=False)
    w1sz = [KH * 128 * e * 128 for e in ecs]
    w1off = np.cumsum([0] + w1sz).tolist()
    w1c_d = nc.declare_dram_parameter("w1c", [w1off[-1]], CDT, isOutput=False)
    w2_d = nc.declare_dram_parameter("w2", [128, W2COLS], CDT, isOutput=False)
    bias_d = nc.declare_dram_parameter("biases", [128, NB], _FP32, isOutput=False)
    if n_mixed:
        mask_d = nc.declare_dram_parameter(
            "mask", [MROWS, n_mixed * CHUNK], _FP32, isOutput=False
        )
    out_d = nc.declare_dram_parameter("out", [BL], _FP32, isOutput=True)

    relu = mybir.ActivationFunctionType.Relu
    copyf = mybir.ActivationFunctionType.Identity

    with TileContext(nc) as tc:
        with (
            tc.tile_pool(name="weights", bufs=1) as wpool,
            tc.tile_pool(name="xin", bufs=16) as xpool,
            tc.tile_pool(name="w1p", bufs=3) as w1pool,
            tc.tile_pool(name="hmid", bufs=3) as hpool,
            tc.tile_pool(name="amid", bufs=3) as apool,
            tc.tile_pool(name="small", bufs=4) as spool,
            tc.tile_pool(name="ps_h", bufs=4, space="PSUM") as ps_h,
            tc.tile_pool(name="ps_a", bufs=2, space="PSUM") as ps_a,
            tc.tile_pool(name="ps_c", bufs=1, space="PSUM") as ps_c,
            tc.tile_pool(name="ps_o", bufs=1, space="PSUM") as ps_o,
        ):
            # ---- DMAs with explicit priorities pinning queue order ----
            _prio = [0]

            def pdma(q, dst, src):
                inst = q.dma_start(dst, src)
                inst.ins.bass_priority = _prio[0]
                _prio[0] += 1
                return inst

            def xpair_view(j, p):
                sz = SIZES[j]
                o = (OFFS[j] * D) + p * (128 * 2 * sz)
                return xT_d[o : o + 128 * 2 * sz].rearrange(
                    "(p i t) -> p i t", p=128, i=2
                )

            def wsh_view(p):
                o = p * (128 * 2 * H)
                return wsh_d[o : o + 128 * 2 * H].rearrange(
                    "(p i h) -> p i h", p=128, i=2
                )

            def w1_view(j):
                return w1c_d[w1off[j] : w1off[j + 1]].rearrange(
                    "(p k c) -> p k c", p=128, k=KH
                )

            # head: wsh pairs + x chunk-0 pairs interleaved on both queues
            wshp = [wpool.tile([128, 2, H], CDT, name=f"wshp{p}") for p in range(NPAIR)]
            xts = [
                [
                    xpool.tile([128, 2, SIZES[j]], CDT, tag="xt", name=f"xt{j}_{p}")
                    for p in range(NPAIR)
                ]
                for j in range(NCH)
            ]
            for p in range(NPAIR):
                qa = nc.sync if p % 2 == 0 else nc.scalar
                qb = nc.scalar if p % 2 == 0 else nc.sync
                pdma(qa, wshp[p][:], wsh_view(p))
                pdma(qb, xts[0][p][:], xpair_view(0, p))

            # small tensors ride the gpsimd software DGE, off the hot rings
            bias_sb = wpool.tile([128, NB], _FP32)
            pdma(nc.gpsimd, bias_sb[:], bias_d[:])
            w2_sb = wpool.tile([128, W2COLS], CDT)
            pdma(nc.gpsimd, w2_sb[:], w2_d[:])
            if n_mixed:
                mask_sb = wpool.tile([MROWS, n_mixed * CHUNK], _FP32)
                pdma(nc.gpsimd, mask_sb[:], mask_d[:])
                ones_sb = wpool.tile([max_ec, 1], CDT)
                nc.vector.memset(ones_sb[:], 1.0)

            w1sb = [None] * NCH
            w1sb[0] = w1pool.tile([128, KH, ecs[0] * 128], CDT, tag="w1", name="w1_0")
            pdma(nc.sync, w1sb[0][:], w1_view(0))
            for j in range(1, NCH):
                for p in range(NPAIR):
                    pdma(
                        nc.sync if p % 2 == 0 else nc.scalar,
                        xts[j][p][:],
                        xpair_view(j, p),
                    )
                w1sb[j] = w1pool.tile(
                    [128, KH, ecs[j] * 128], CDT, tag="w1", name=f"w1_{j}"
                )
                pdma(nc.scalar if j % 2 else nc.sync, w1sb[j][:], w1_view(j))

            # ---- compute ----
            w2c = 0          # running col offset into w2_sb
            bcol = MH        # running col offset into bias_sb
            mj = 0           # mixed-chunk ordinal
            for j in range(NCH):
                ec = ecs[j]
                sz = SIZES[j]
                xt = xts[j]

                # M1: hT = relu(W_shared.T @ xT + b)
                hT = hpool.tile([128, MH, sz], CDT, tag="hT", name=f"hT{j}")
                if j <= 1:
                    # k-outer so matmuls start as the first pair DMAs land
                    phs = [
                        ps_h.tile([128, sz], _FP32, tag="ps_h", name=f"ph{j}_{m}")
                        for m in range(MH)
                    ]
                    for p in range(NPAIR):
                        for i in range(2):
                            for m in range(MH):
                                nc.tensor.matmul(
                                    phs[m][:],
                                    lhsT=wshp[p][:, i, m * 128 : (m + 1) * 128],
                                    rhs=xt[p][:, i, :],
                                    start=(p == 0 and i == 0),
                                    stop=(p == NPAIR - 1 and i == 1),
                                )
                    for m in range(MH):
                        nc.scalar.activation(
                            hT[:, m, :], phs[m][:], relu, bias=bias_sb[:, m : m + 1]
                        )
                else:
                    for m in range(MH):
                        ph = ps_h.tile(
                            [128, sz], _FP32, tag="ps_h", name=f"ph{j}_{m}"
                        )
                        for p in range(NPAIR):
                            for i in range(2):
                                nc.tensor.matmul(
                                    ph[:],
                                    lhsT=wshp[p][:, i, m * 128 : (m + 1) * 128],
                                    rhs=xt[p][:, i, :],
                                    start=(p == 0 and i == 0),
                                    stop=(p == NPAIR - 1 and i == 1),
                                )
                        nc.scalar.activation(
                            hT[:, m, :], ph[:], relu, bias=bias_sb[:, m : m + 1]
                        )

                # M2: aT = relu(W1sel.T @ hT + b1)
                aT = apool.tile([128, ec, sz], CDT, tag="aT", name=f"aT{j}")
                for mi in range(ec):
                    pa = ps_a.tile([128, sz], _FP32, tag="ps_a", name=f"pa{j}_{mi}")
                    for k in range(KH):
                        nc.tensor.matmul(
                            pa[:],
                            lhsT=w1sb[j][:, k, mi * 128 : (mi + 1) * 128],
                            rhs=hT[:, k, :],
                            start=(k == 0),
                            stop=(k == KH - 1),
                        )
                    nc.scalar.activation(
                        aT[:, mi, :], pa[:], relu,
                        bias=bias_sb[:, bcol + mi : bcol + mi + 1],
                    )

                # M3 + select
                t0 = OFFS[j]
                ot = spool.tile([1, sz], _FP32, tag="ot", name=f"ot{j}")
                if not mixed[j]:
                    po = ps_o.tile([1, sz], _FP32, tag="ps_o", name=f"po{j}")
                    for k in range(ec):
                        nc.tensor.matmul(
                            po[:],
                            lhsT=w2_sb[:, w2c + k : w2c + k + 1],
                            rhs=aT[:, k, :],
                            start=(k == 0),
                            stop=(k == ec - 1),
                        )
                    nc.scalar.activation(
                        ot[:], po[:], copyf,
                        bias=bias_sb[0:1, bcol + ec : bcol + ec + 1],
                    )
                    w2c += ec
                    bcol += ec + 1
                else:
                    pc = ps_c.tile([ec, sz], _FP32, tag="ps_c", name=f"pc{j}")
                    for k in range(ec):
                        nc.tensor.matmul(
                            pc[:],
                            lhsT=w2_sb[:, w2c + k * ec : w2c + (k + 1) * ec],
                            rhs=aT[:, k, :],
                            start=(k == 0),
                            stop=(k == ec - 1),
                        )
                    msel = spool.tile([ec, sz], CDT, tag="msel", name=f"msel{j}")
                    nc.vector.tensor_mul(
                        msel[:], pc[:],
                        mask_sb[:ec, mj * CHUNK : mj * CHUNK + sz],
                    )
                    po = ps_o.tile([1, sz], _FP32, tag="ps_o", name=f"pom{j}")
                    nc.tensor.matmul(
                        po[:], lhsT=ones_sb[:ec, :], rhs=msel[:], start=True, stop=True
                    )
                    nc.vector.tensor_add(
                        ot[:], po[:],
                        mask_sb[32:33, mj * CHUNK : mj * CHUNK + sz],
                    )
                    w2c += ec * ec
                    bcol += ec
                    mj += 1
                nc.gpsimd.dma_start(
                    out_d[t0 : t0 + sz].rearrange("(o t) -> o t", o=1), ot[:]
                )

    nc.compile()
    return nc


def get_nc(key):
    if key not in _cache:
        _cache[key] = _build_nc(key)
    return _cache[key]


def prepare(inputs):
    """Host-side routing/sorting/sharding. Returns (key, in_maps, perm)."""
    x = np.asarray(inputs["x"], dtype=np.float32)
    idx = np.asarray(inputs["idx"]).astype(np.int64).reshape(B)
    W_shared = np.asarray(inputs["W_shared"], dtype=np.float32)
    b_shared = np.asarray(inputs["b_shared"], dtype=np.float32).reshape(H)
    W1 = np.asarray(inputs["W1"], dtype=np.float32)
    b1 = np.asarray(inputs["b1"], dtype=np.float32).reshape(E, F)
    W2 = np.asarray(inputs["W2"], dtype=np.float32).reshape(E, F)
    b2 = np.asarray(inputs["b2"], dtype=np.float32).reshape(E)
    send_to = np.asarray(inputs["send_to"]).astype(np.int64)

    perm = np.argsort(idx, kind="stable")
    idx_s = idx[perm]
    x_s = x[perm]
    routes_s = send_to[idx_s]                      # [B, K] sorted routes

    # per-position structure: slot count + masked?, uniform across cores
    slot_lists = [[None] * NCH for _ in range(N_CORES)]
    ecs, mixed = [], []
    for j in range(NCH):
        ec_j, mx_j = 2, False
        for c in range(N_CORES):
            sl = slice(c * BL + OFFS[j], c * BL + OFFS[j + 1])
            experts = np.unique(routes_s[sl])
            slot_lists[c][j] = experts
            ec_j = max(ec_j, len(experts))
            if len(np.unique(idx_s[sl])) > 1:
                mx_j = True
        ecs.append(ec_j)
        mixed.append(mx_j)
    ecs, mixed = tuple(ecs), tuple(mixed)
    n_mixed = sum(mixed)
    max_ec = max(ecs)
    MROWS = 33

    # wsh pair blocks [pair, 128, 2, H]
    wshr = W_shared.reshape(NPAIR, 2, 128, H)
    wsh_flat = np.ascontiguousarray(wshr.transpose(0, 2, 1, 3)).astype(NP_CDT).ravel()

    in_maps = []
    for c in range(N_CORES):
        xc = x_s[c * BL : (c + 1) * BL]
        # per-chunk pair blocks [pair, 128, 2, sz]
        xparts = []
        for j in range(NCH):
            xj = xc[OFFS[j] : OFFS[j + 1]].reshape(SIZES[j], NPAIR, 2, 128)
            xparts.append(
                np.ascontiguousarray(xj.transpose(1, 3, 2, 0)).astype(NP_CDT).ravel()
            )
        xT = np.concatenate(xparts)

        w1_parts = []
        w2_cols = []
        bias_cols = [b_shared.reshape(MH, 128).T]
        mask_cols = np.zeros((MROWS, max(n_mixed, 1) * CHUNK), np.float32)
        mj = 0
        for j in range(NCH):
            sl = slice(c * BL + OFFS[j], c * BL + OFFS[j + 1])
            ec = ecs[j]
            slots = np.full(ec, -1, dtype=np.int64)
            el = slot_lists[c][j]
            slots[: len(el)] = el

            w1sel = np.zeros((H, ec * 128), np.float32)
            b1sel = np.zeros(ec * 128, np.float32)
            for mi, e in enumerate(slots):
                if e < 0:
                    continue
                w1sel[:, mi * 128 : mi * 128 + F] = W1[e]
                b1sel[mi * 128 : mi * 128 + F] = b1[e]
            w1_parts.append(
                np.ascontiguousarray(
                    w1sel.reshape(KH, 128, ec * 128).transpose(1, 0, 2)
                ).astype(NP_CDT).ravel()
            )
            bias_cols.append(b1sel.reshape(ec, 128).T)

            r = routes_s[sl]                        # [CHUNK, K]
            if not mixed[j]:
                w2m = np.zeros((128, ec), np.float32)
                for e in r[0]:  # routes with multiplicity
                    mi = int(np.where(slots == e)[0][0])
                    w2m[:F, mi] += W2[e] / r.shape[1]
                w2_cols.append(w2m)
                col = np.zeros((128, 1), np.float32)
                col[0, 0] = b2[r[0]].mean()
                bias_cols.append(col)
            else:
                w2full = np.zeros((ec * 128, ec), np.float32)
                for mi, e in enumerate(slots):
                    if e < 0:
                        continue
                    w2full[mi * 128 : mi * 128 + F, mi] = W2[e]
                w2_cols.append(
                    w2full.reshape(ec, 128, ec).transpose(1, 0, 2).reshape(128, ec * ec)
                )
                sz = SIZES[j]
                for k in range(r.shape[1]):
                    hit = slots[:, None] == r[None, :, k]
                    mask_cols[:ec, mj * CHUNK : mj * CHUNK + sz] += (
                        hit.astype(np.float32) / r.shape[1]
                    )
                mask_cols[32, mj * CHUNK : mj * CHUNK + sz] = b2[r].mean(axis=1)
                mj += 1

        in_map = {
            "xT": xT,
            "wsh": wsh_flat,
            "w1c": np.concatenate(w1_parts),
            "w2": np.concatenate(w2_cols, axis=1).astype(NP_CDT),
            "biases": np.ascontiguousarray(
                np.concatenate(bias_cols, axis=1)
            ).astype(np.float32),
        }
        if n_mixed:
            in_map["mask"] = mask_cols
        in_maps.append(in_map)
    return (ecs, mixed), in_maps, perm


def kernel(**inputs) -> np.ndarray:
    key, in_maps, perm = prepare(inputs)
    nc = get_nc(key)
    res = run_bass_kernel_spmd(nc, in_maps, list(range(N_CORES)))
    out_sorted = np.concatenate([res.results[c]["out"] for c in range(N_CORES)])
    out = np.empty(B, dtype=np.float32)
    out[perm] = out_sorted
    return out.reshape(B, 1)
